# revision 2
# baseline (speedup 1.0000x reference)
"""Multi-head attention (B=2, S=2048, D=1024, H=16) on 8 trn2 NeuronCores.

Sharding: core c handles batch b=c//4 and query rows [512*(c%4), +512).

Key compaction: the mask zeroes ~half the key positions outright, so the
host gathers only the kept keys (plus zero padding up to C, a multiple of
128) and the kernel runs attention over C keys instead of S=2048. Padded
keys get an exp bias of -1e5 so they contribute exactly 0 to both the
numerator and the softmax denominator.

K/V projection is sharded across the 4 cores of each batch group (core
projects its C/4-key chunk only), then exchanged with two sub-1MB
AllGathers (Mesh algorithm). Attention accumulates the unnormalized
head outputs plus a denominator row directly in PSUM across all key
tiles of a head pair (V carries a ones column), then normalizes via a
PE-transposed reciprocal (d -> [128,8] layout so the DVE's 8-cycle/elem
reciprocal runs on 128 lanes instead of 1).

Layouts are feature-major so no on-chip transposes are needed:
  - inputs passed as query.T/key.T/value.T [D, rows], weights as W.T
  - scores computed transposed [k, q]; softmax across k (partitions):
    exp on ACT with mask folded into the per-partition exp bias,
    denominator via a ones-column appended to V in the attn@V matmul
  - 1/sqrt(dk) folded into wq host-side; bv/bo folded into bo+wo@bv
"""

import sys

for _p in ("/opt/trn_rl_repo", "/root/.axon_site/_ro/trn_rl_repo"):
    if _p not in sys.path:
        sys.path.insert(0, _p)

import numpy as np
import ml_dtypes

B, S, D, H, DK = 2, 2048, 1024, 16, 64
NCORES = 8
MQ = 512          # query rows per core
P = 128
NIT = D // P      # 8 input-feature tiles
NOT_ = D // P     # 8 output-feature tiles
NJ = H // 2       # 8 head pairs
VW = DK + 1       # 65: head dim + ones column
VCOLS = H * VW    # 1040

BF16 = ml_dtypes.bfloat16

_CACHE = {}


def _build(C):
    from concourse import bacc
    import concourse.mybir as mybir
    import concourse.tile as tile
    import concourse.bass as bass

    NKT = C // P            # key tiles
    C4 = C // 4             # keys projected per core
    VKT = (C4 + P - 1) // P # local V key tiles
    RL = C4 - P * (VKT - 1) # rows in the last local V tile
    KFLAT = D * C4
    VFLAT = C4 * VCOLS

    nc = bacc.Bacc("TRN2", target_bir_lowering=False, debug=False)
    dt = mybir.dt

    qT = nc.dram_tensor("qT", [D, MQ], dt.bfloat16, kind="ExternalInput")
    kT = nc.dram_tensor("kT", [D, C4], dt.bfloat16, kind="ExternalInput")
    vT = nc.dram_tensor("vT", [D, C4], dt.bfloat16, kind="ExternalInput")
    wq = nc.dram_tensor("wq", [D, D], dt.bfloat16, kind="ExternalInput")
    wk = nc.dram_tensor("wk", [D, D], dt.bfloat16, kind="ExternalInput")
    wv = nc.dram_tensor("wv", [D, D], dt.bfloat16, kind="ExternalInput")
    wo = nc.dram_tensor("wo", [D, D], dt.bfloat16, kind="ExternalInput")
    bq = nc.dram_tensor("bq", [P, NOT_], dt.float32, kind="ExternalInput")
    bk = nc.dram_tensor("bk", [P, NOT_], dt.float32, kind="ExternalInput")
    maskb = nc.dram_tensor("maskb", [P, NKT], dt.float32, kind="ExternalInput")
    bob = nc.dram_tensor("bob", [1, D], dt.float32, kind="ExternalInput")
    ones64 = nc.dram_tensor("ones64", [1, DK], dt.float32r, kind="ExternalInput")
    ident = nc.dram_tensor("ident", [P, P], dt.float32r, kind="ExternalInput")
    onec = nc.dram_tensor("onec", [1, 1], dt.float32, kind="ExternalInput")
    out = nc.dram_tensor("out", [MQ, D], dt.float32, kind="ExternalOutput")

    agk_in = nc.dram_tensor("agk_in", [KFLAT], dt.bfloat16)
    agk_out = nc.dram_tensor("agk_out", [4 * KFLAT], dt.bfloat16)
    agv_in = nc.dram_tensor("agv_in", [VFLAT], dt.bfloat16)
    agv_out = nc.dram_tensor("agv_out", [4 * VFLAT], dt.bfloat16)

    with tile.TileContext(nc) as tc:
        with (
            tc.tile_pool(name="w", bufs=3) as wpool,
            tc.tile_pool(name="stat", bufs=1) as stat,
            tc.tile_pool(name="inT", bufs=2) as inpool,
            tc.tile_pool(name="qin", bufs=1) as qin,
            tc.tile_pool(name="loc", bufs=1) as loc,
            tc.tile_pool(name="all_", bufs=1) as allp,
            tc.tile_pool(name="qt", bufs=1) as qtp,
            tc.tile_pool(name="ctx", bufs=1) as ctxp,
            tc.tile_pool(name="pp", bufs=12) as pp,
            tc.tile_pool(name="avs", bufs=2) as avs,
            tc.tile_pool(name="rs", bufs=2) as rs,
            tc.tile_pool(name="outp", bufs=2) as outp,
            tc.tile_pool(name="psS", bufs=2, space="PSUM") as psS,
            tc.tile_pool(name="psAV", bufs=1, space="PSUM") as psAV,
            tc.tile_pool(name="psN", bufs=1, space="PSUM") as psN,
        ):
            # ---- constants ----
            bq_sb = stat.tile([P, NOT_], dt.float32, tag="bq")
            bk_sb = stat.tile([P, NOT_], dt.float32, tag="bk")
            mb_sb = stat.tile([P, NKT], dt.float32, tag="mb")
            bob_sb = stat.tile([P, D], dt.float32, tag="bob")
            ones_sb = stat.tile([1, DK], dt.float32r, tag="ones")
            id_sb = stat.tile([P, P], dt.float32r, tag="ident")
            onec_sb = stat.tile([1, 1], dt.float32, tag="onec")
            nc.sync.dma_start(out=bq_sb, in_=bq[:, :])
            nc.sync.dma_start(out=bk_sb, in_=bk[:, :])
            nc.sync.dma_start(out=mb_sb, in_=maskb[:, :])
            bob_bcast = bass.AP(
                tensor=bob.ap().tensor, offset=0, ap=[[0, P], [1, D]]
            )
            nc.sync.dma_start(out=bob_sb, in_=bob_bcast)
            nc.sync.dma_start(out=ones_sb, in_=ones64[:, :])
            nc.sync.dma_start(out=id_sb, in_=ident[:, :])
            nc.sync.dma_start(out=onec_sb, in_=onec[:, :])

            def load_w(name, dram):
                t = wpool.tile([P, NIT, D], dt.bfloat16, tag="w", name=name)
                nc.sync.dma_start(
                    out=t, in_=dram.ap().rearrange("(t p) o -> p t o", p=P)
                )
                return t

            # ---- K projection (local chunk) + AllGather ----
            wk_sb = load_w("wk_sb", wk)
            kTl = inpool.tile([P, NIT, C4], dt.bfloat16, tag="inT", name="kTl")
            nc.sync.dma_start(
                out=kTl, in_=kT.ap().rearrange("(t p) k -> p t k", p=P)
            )
            KTl = loc.tile([P, NOT_, C4], dt.bfloat16, tag="KTl")
            for ot in range(NOT_):
                ps = psS.tile([P, C4], dt.float32, tag="sc", name=f"psk{ot}")
                for it in range(NIT):
                    nc.tensor.matmul(
                        ps,
                        lhsT=wk_sb[:, it, ot * P : (ot + 1) * P],
                        rhs=kTl[:, it, :],
                        start=(it == 0),
                        stop=(it == NIT - 1),
                    )
                nc.vector.tensor_scalar_add(
                    out=KTl[:, ot, :], in0=ps, scalar1=bk_sb[:, ot : ot + 1]
                )
            nc.sync.dma_start(
                out=agk_in.ap().rearrange("(t p k) -> p t k", p=P, k=C4),
                in_=KTl,
            )
            nc.gpsimd.collective_compute(
                "AllGather",
                mybir.AluOpType.bypass,
                ins=[agk_in[:]],
                outs=[agk_out[:]],
                replica_groups=[[0, 1, 2, 3], [4, 5, 6, 7]],
            )

            # ---- V projection (local chunk) + AllGather ----
            wv_sb = load_w("wv_sb", wv)
            vTl = inpool.tile([P, NIT, C4], dt.bfloat16, tag="inT", name="vTl")
            nc.sync.dma_start(
                out=vTl, in_=vT.ap().rearrange("(t p) k -> p t k", p=P)
            )
            Vpl = loc.tile([P, VKT, VCOLS], dt.bfloat16, tag="Vpl")
            vones = Vpl.rearrange("p t (h x) -> p t h x", x=VW)[:, :, :, DK : DK + 1]
            nc.vector.memset(vones, 1.0)
            for kt in range(VKT):
                rows = P if kt < VKT - 1 else RL
                for oc in range(2):
                    ps = psS.tile(
                        [P, 512], dt.float32, tag="sc", name=f"psv{kt}_{oc}"
                    )
                    for it in range(NIT):
                        nc.tensor.matmul(
                            ps[0:rows, :],
                            lhsT=vTl[:, it, kt * P : kt * P + rows],
                            rhs=wv_sb[:, it, oc * 512 : (oc + 1) * 512],
                            start=(it == 0),
                            stop=(it == NIT - 1),
                        )
                    dst = Vpl[:, kt, oc * 8 * VW : (oc * 8 + 8) * VW].rearrange(
                        "p (h x) -> p h x", x=VW
                    )[0:rows, :, 0:DK]
                    nc.vector.tensor_copy(
                        out=dst,
                        in_=ps[0:rows, :].rearrange("p (h x) -> p h x", x=DK),
                    )
            off = 0
            for kt in range(VKT):
                rows = P if kt < VKT - 1 else RL
                nc.sync.dma_start(
                    out=agv_in[off : off + rows * VCOLS].rearrange(
                        "(p c) -> p c", c=VCOLS
                    ),
                    in_=Vpl[0:rows, kt, :],
                )
                off += rows * VCOLS
            nc.gpsimd.collective_compute(
                "AllGather",
                mybir.AluOpType.bypass,
                ins=[agv_in[:]],
                outs=[agv_out[:]],
                replica_groups=[[0, 1, 2, 3], [4, 5, 6, 7]],
            )

            # ---- Q projection (overlaps the AllGathers) ----
            wq_sb = load_w("wq_sb", wq)
            qT_sb = qin.tile([P, NIT, MQ], dt.bfloat16, tag="qTin")
            nc.sync.dma_start(
                out=qT_sb, in_=qT.ap().rearrange("(t p) q -> p t q", p=P)
            )
            QT_sb = qtp.tile([P, NOT_, MQ], dt.bfloat16, tag="QT")
            for ot in range(NOT_):
                ps = psS.tile([P, MQ], dt.float32, tag="sc", name=f"psq{ot}")
                for it in range(NIT):
                    nc.tensor.matmul(
                        ps,
                        lhsT=wq_sb[:, it, ot * P : (ot + 1) * P],
                        rhs=qT_sb[:, it, :],
                        start=(it == 0),
                        stop=(it == NIT - 1),
                    )
                nc.vector.tensor_scalar_add(
                    out=QT_sb[:, ot, :], in0=ps, scalar1=bq_sb[:, ot : ot + 1]
                )

            # ---- gathered K/V into SBUF ----
            KT_all = allp.tile([P, NOT_, C], dt.bfloat16, tag="KTall")
            for ci in range(4):
                nc.sync.dma_start(
                    out=KT_all[:, :, ci * C4 : (ci + 1) * C4],
                    in_=agk_out[ci * KFLAT : (ci + 1) * KFLAT].rearrange(
                        "(t p k) -> p t k", p=P, k=C4
                    ),
                )
            V_all = allp.tile([P, NKT, VCOLS], dt.bfloat16, tag="Vall")
            nc.sync.dma_start(
                out=V_all,
                in_=agv_out.ap().rearrange("(t p c) -> p t c", p=P, c=VCOLS),
            )

            wo_sb = load_w("wo_sb", wo)
            ctx_sb = ctxp.tile([P, NOT_, MQ], dt.bfloat16, tag="ctx")

            # ---- attention: per head pair, PSUM-resident accumulation ----
            for j in range(NJ):
                av = psAV.tile([VW, 1024], dt.float32, tag="av", name=f"av{j}")
                for kt in range(NKT):
                    sc = psS.tile(
                        [P, 1024], dt.float32, tag="sc", name=f"sc{j}_{kt}"
                    )
                    nc.tensor.matmul(
                        sc[:, 0:512],
                        lhsT=KT_all[0:DK, j, kt * P : (kt + 1) * P],
                        rhs=QT_sb[0:DK, j, :],
                        start=True,
                        stop=True,
                        tile_position=(0, 0),
                    )
                    nc.tensor.matmul(
                        sc[:, 512:1024],
                        lhsT=KT_all[DK:P, j, kt * P : (kt + 1) * P],
                        rhs=QT_sb[DK:P, j, :],
                        start=True,
                        stop=True,
                        tile_position=(DK, 0),
                    )
                    p_kt = pp.tile([P, 1024], dt.bfloat16, tag="pT")
                    nc.scalar.activation(
                        out=p_kt,
                        in_=sc,
                        func=mybir.ActivationFunctionType.Exp,
                        bias=mb_sb[:, kt : kt + 1],
                        scale=1.0,
                    )
                    for hh in range(2):
                        nc.tensor.matmul(
                            av[:, hh * 512 : (hh + 1) * 512],
                            lhsT=V_all[
                                :, kt, (2 * j + hh) * VW : (2 * j + hh + 1) * VW
                            ],
                            rhs=p_kt[:, hh * 512 : (hh + 1) * 512],
                            start=(kt == 0),
                            stop=(kt == NKT - 1),
                            skip_group_check=True,
                        )
                # normalize: ctx_h = av[0:64] / av[64]
                av_sb = avs.tile([VW, 1024], dt.float32, tag="avsb")
                nc.vector.tensor_copy(out=av_sb, in_=av)
                dT = psN.tile([P, 8], dt.float32, tag="nm", name=f"dT{j}")
                for b in range(8):
                    nc.tensor.matmul(
                        dT[:, b : b + 1],
                        lhsT=av_sb[DK : DK + 1, b * P : (b + 1) * P],
                        rhs=onec_sb,
                        start=True,
                        stop=True,
                    )
                rT = rs.tile([P, 8], dt.float32r, tag="rT")
                with nc.allow_low_precision(
                    reason="fp32r keeps most of the mantissa"
                ):
                    nc.vector.reciprocal(out=rT, in_=dT)
                rps = psN.tile([8, P], dt.float32, tag="nm", name=f"rps{j}")
                nc.tensor.matmul(rps, lhsT=rT, rhs=id_sb, start=True, stop=True)
                r_sb = rs.tile([8, P], dt.float32r, tag="rrow")
                nc.vector.tensor_copy(out=r_sb, in_=rps)
                bc = psN.tile([DK, 1024], dt.float32, tag="nm", name=f"bc{j}")
                for b in range(8):
                    nc.tensor.matmul(
                        bc[:, b * P : (b + 1) * P],
                        lhsT=ones_sb,
                        rhs=r_sb[b : b + 1, :],
                        start=True,
                        stop=True,
                    )
                nc.vector.tensor_mul(
                    out=ctx_sb[0:DK, j, :],
                    in0=av_sb[0:DK, 0:512],
                    in1=bc[:, 0:512],
                )
                nc.vector.tensor_mul(
                    out=ctx_sb[DK:P, j, :],
                    in0=av_sb[0:DK, 512:1024],
                    in1=bc[:, 512:1024],
                )

            # ---- output projection ----
            for qt in range(MQ // P):
                for oc in range(2):
                    ps = psS.tile(
                        [P, 512], dt.float32, tag="sc", name=f"pso{qt}_{oc}"
                    )
                    for jt in range(NJ):
                        nc.tensor.matmul(
                            ps,
                            lhsT=ctx_sb[:, jt, qt * P : (qt + 1) * P],
                            rhs=wo_sb[:, jt, oc * 512 : (oc + 1) * 512],
                            start=(jt == 0),
                            stop=(jt == NJ - 1),
                        )
                    o_sb = outp.tile([P, 512], dt.float32, tag="osb")
                    nc.vector.tensor_add(
                        out=o_sb,
                        in0=ps,
                        in1=bob_sb[:, oc * 512 : (oc + 1) * 512],
                    )
                    nc.sync.dma_start(
                        out=out[qt * P : (qt + 1) * P, oc * 512 : (oc + 1) * 512],
                        in_=o_sb,
                    )

    nc.finalize()
    return nc


def _get_nc(C):
    if C not in _CACHE:
        _CACHE[C] = _build(C)
    return _CACHE[C]


def _make_inputs(query, key, value, mask, wq, bq, wk, bk, wv, bv, wo, bo):
    f32 = np.float32
    query = np.asarray(query, dtype=f32)
    key = np.asarray(key, dtype=f32)
    value = np.asarray(value, dtype=f32)
    mask = np.asarray(mask)

    # key compaction
    idx = [np.nonzero(mask[b, 0, 0] != 0)[0] for b in range(B)]
    nmax = max(max(len(i) for i in idx), 1)
    C = ((nmax + P - 1) // P) * P
    C4 = C // 4
    NKT = C // P

    keyc = np.zeros((B, C, D), dtype=f32)
    valc = np.zeros((B, C, D), dtype=f32)
    mbias = np.zeros((B, C), dtype=f32)
    for b in range(B):
        n = len(idx[b])
        keyc[b, :n] = key[b][idx[b]]
        valc[b, :n] = value[b][idx[b]]
        mbias[b, n:] = -1e5

    wqT = np.ascontiguousarray(np.asarray(wq, f32).T / 8.0).astype(BF16)
    wkT = np.ascontiguousarray(np.asarray(wk, f32).T).astype(BF16)
    wvT = np.ascontiguousarray(np.asarray(wv, f32).T).astype(BF16)
    woT = np.ascontiguousarray(np.asarray(wo, f32).T).astype(BF16)
    bq8 = np.ascontiguousarray((np.asarray(bq, f32) / 8.0).reshape(NOT_, P).T)
    bkr = np.ascontiguousarray(np.asarray(bk, f32).reshape(NOT_, P).T)
    bob = (np.asarray(bo, f32) + np.asarray(wo, f32) @ np.asarray(bv, f32))[None, :]
    bob = np.ascontiguousarray(bob)
    ones64 = np.ones((1, DK), dtype=f32)
    ident = np.eye(P, dtype=f32)
    onec = np.ones((1, 1), dtype=f32)

    in_maps = []
    for c in range(NCORES):
        b = c // 4
        L = c % 4
        q0 = L * MQ
        qTc = np.ascontiguousarray(query[b].T[:, q0 : q0 + MQ]).astype(BF16)
        kTc = np.ascontiguousarray(keyc[b].T[:, L * C4 : (L + 1) * C4]).astype(BF16)
        vTc = np.ascontiguousarray(valc[b].T[:, L * C4 : (L + 1) * C4]).astype(BF16)
        mb = np.ascontiguousarray(mbias[b].reshape(NKT, P).T)
        in_maps.append(
            {
                "qT": qTc,
                "kT": kTc,
                "vT": vTc,
                "wq": wqT,
                "wk": wkT,
                "wv": wvT,
                "wo": woT,
                "bq": bq8,
                "bk": bkr,
                "maskb": mb,
                "bob": bob,
                "ones64": ones64,
                "ident": ident,
                "onec": onec,
            }
        )
    return C, in_maps


def kernel(query, key, value, mask, wq, bq, wk, bk, wv, bv, wo, bo):
    from concourse.bass_utils import run_bass_kernel_spmd

    C, in_maps = _make_inputs(
        query, key, value, mask, wq, bq, wk, bk, wv, bv, wo, bo
    )
    nc = _get_nc(C)
    res = run_bass_kernel_spmd(nc, in_maps, core_ids=list(range(NCORES)))
    out = np.empty((B, S, D), dtype=np.float32)
    for c in range(NCORES):
        b = c // 4
        q0 = (c % 4) * MQ
        out[b, q0 : q0 + MQ, :] = res.results[c]["out"]
    return out
